# revision 25
# baseline (speedup 1.0000x reference)
"""Dense SE(3) Gauss-Newton kernel for Trainium2, sharded over 8 NeuronCores.

Sharding: core owns batch b = core//4 and a 256-anchor slab of the i axis;
the k axis (1024) runs in 8 chunks of 128 on the partition dimension with
anchors on the free dimension.

Per (i,k) the kernel materializes 12 fp16 band surfaces (powers of the
projected-point deltas d, dX, dY, optionally weighted by the embedding
affinity); the 6x6 normal equations + rhs are accumulated straight into
PSUM as 27 entry rows by matmuls against host-precomputed per-k fp16
coefficient tables.  The residual is decomposed around each point's
self-projection (delta form) so every band is cancellation-free and
fp16-safe; the dkk self-projection term is folded into the stationary
(d^3 and d^2 get separate coefficient columns) so no f32 band survives.
Geometry per chunk is ONE 1024-wide matmul (X'|Y'|Z|s); 1/Z runs on the
Vector engine; band products are half-kernel-wide flat f16 Vector ops;
relu/f16-conversions run on GpSimd; the ACT engine only does
sqrt/exp for the affinity (2 tables).  PE is pre-ramped with dummy
matmuls during the input-DMA window; input DMAs issue from three queues
in parallel.
"""
import sys

sys.path.insert(0, "/opt/trn_rl_repo")

import numpy as np

from concourse import bacc, tile
import concourse.mybir as mybir
from concourse.bass_utils import run_bass_kernel_spmd

F32 = mybir.dt.float32
F16 = mybir.dt.float16
AF = mybir.ActivationFunctionType
ALU = mybir.AluOpType
AX = mybir.AxisListType

B, C, H, W = 2, 16, 32, 32
N = H * W
NCORES = 8
SLAB = 256
KC = 8
P = 128
GR = 32                       # geometry contraction rows
HTRI = [(p, q) for p in range(6) for q in range(p, 6)]  # 21 entries
NHB = 6                       # Hm bands: A, AdX, AdY, AdX2, AdY2, Ad2
NRB = 6                       # rhs bands: ddX, ddY, ddX2, ddY2, d3, d2
ACC_CW = NHB * 21 + NRB * 6   # 162 stationary cols per chunk
NE = 48                       # augmented 6x7 system padded
HALF = 4 * SLAB               # 1024: four chunks of s/d columns


def build_nc():
    nc = bacc.Bacc("TRN2", target_bir_lowering=False, debug=False)

    geom_d = nc.dram_tensor("geom", [GR, 2048], F16, kind="ExternalInput")
    accst_d = nc.dram_tensor("accst", [P, KC * ACC_CW], F16, kind="ExternalInput")
    misc_d = nc.dram_tensor("misc", [P, 128], F32, kind="ExternalInput")
    out_d = nc.dram_tensor("out", [P, 32], F32, kind="ExternalOutput")

    with tile.TileContext(nc) as tc:
        with tc.tile_pool(name="persist", bufs=1) as pp, \
             tc.tile_pool(name="acc_ps", bufs=1, space="PSUM") as accp:

            geom = pp.tile([GR, 2048], F16)
            accst = pp.tile([P, KC * ACC_CW], F16)
            misc = pp.tile([P, 128], F32)
            sqpre = pp.tile([P, 1], F16)

            # ---- PE pre-ramp dummies (run while input DMAs are in flight)
            # plus a tiny Sqrt to pull the first ACT table load off the
            # critical path
            with tc.tile_pool(name="ramp", bufs=1) as rp, \
                 tc.tile_pool(name="ramp_ps", bufs=1, space="PSUM") as rps:
                dsm = rp.tile([GR, 640], F16)
                nc.vector.memset(dsm[:], 0.0)
                nc.vector.memset(sqpre[:], 1.0)
                nc.scalar.activation(sqpre[:], sqpre[:], AF.Sqrt)
                dp = rps.tile([P, 512], F32)
                for _ in range(4):
                    nc.tensor.matmul(dp[:], dsm[:, 0:128], dsm[:, 128:640],
                                     start=True, stop=True)

            # ---- input DMAs, three queues in parallel; first-needed first
            nc.sync.dma_start(geom[:, 1536:2048], geom_d[:, 1536:2048])
            nc.sync.dma_start(geom[:, 0:128], geom_d[:, 0:128])
            nc.gpsimd.dma_start(geom[:, 1024:1536], geom_d[:, 1024:1536])
            nc.gpsimd.dma_start(geom[:, 128:1024], geom_d[:, 128:1024])
            nc.scalar.dma_start(accst[:], accst_d[:])
            nc.scalar.dma_start(misc[:], misc_d[:])
            stat = geom[:, 0:1024]
            mov = geom[:, 1024:2048]

            sallh = pp.tile([P, 2048], F16)   # ||e_i-e_k||^2, relu'd
            atmp = pp.tile([P, 2048], F16)    # sqrt(s)
            affh = pp.tile([P, 2048], F16)    # exp(-||e_i-e_k||)
            dhall = pp.tile([P, 2048], F16)   # d = 1/Zp
            d2all = pp.tile([P, 2048], F16)
            d3all = pp.tile([P, 2048], F16)
            dXYall = pp.tile([P, 4096], F16)  # [dX | dY] per chunk, 512 wide
            bdXY = pp.tile([P, 4096], F16)
            bdXY2 = pp.tile([P, 4096], F16)
            bAall = pp.tile([P, 2048], F16)
            bAd2 = pp.tile([P, 2048], F16)
            bAdXY = pp.tile([P, 4096], F16)
            bAdXY2 = pp.tile([P, 4096], F16)

            accH = accp.tile([21, SLAB], F32)
            accR = accp.tile([6, SLAB], F32)

            with tc.tile_pool(name="mm_ps", bufs=2, space="PSUM") as mmp, \
                 tc.tile_pool(name="work", bufs=2) as wp:

                dt_pair = [None]

                def pass_s(c):
                    # embedding-distance matmul for chunk c (feeds affinity)
                    ck = slice(c * P, (c + 1) * P)
                    cs = slice(c * SLAB, (c + 1) * SLAB)
                    sq = mmp.tile([P, SLAB], F32, name=f"sq{c}", tag="sq")
                    nc.tensor.matmul(sq[:], stat[:, ck], mov[:, 768:1024],
                                     start=True, stop=True)
                    nc.scalar.activation(sallh[:, cs], sq[:], AF.Relu)

                def pass_zxy(c):
                    # point-transform matmuls for chunk c: Z then [X'|Y']
                    ck = slice(c * P, (c + 1) * P)
                    cs = slice(c * SLAB, (c + 1) * SLAB)
                    c2 = slice(c * 2 * SLAB, (c + 1) * 2 * SLAB)
                    zp = mmp.tile([P, SLAB], F32, name=f"zp{c}", tag="zp")
                    xy = mmp.tile([P, 512], F32, name=f"xy{c}", tag="xy")
                    nc.tensor.matmul(zp[:], stat[:, ck], mov[:, 512:768],
                                     start=True, stop=True)
                    nc.tensor.matmul(xy[:], stat[:, ck], mov[:, 0:512],
                                     start=True, stop=True)
                    if c % 2 == 0:
                        dt_pair[0] = wp.tile([P, 512], F32, name=f"dt{c}",
                                             tag="dt")
                    dt_ = dt_pair[0][:, (c % 2) * SLAB : (c % 2 + 1) * SLAB]
                    nc.vector.reciprocal_approx_fast(dt_, zp[:])
                    nc.vector.tensor_tensor(
                        dXYall[:, c2].rearrange("p (b n) -> p b n", b=2),
                        xy[:].rearrange("p (b n) -> p b n", b=2),
                        dt_.unsqueeze(1).to_broadcast((P, 2, SLAB)), ALU.mult)
                    if c % 2 == 1:
                        nc.gpsimd.tensor_copy(
                            dhall[:, (c - 1) * SLAB : (c + 1) * SLAB],
                            dt_pair[0][:])

                def aff_sqrt(c0, nch):
                    hs = slice(c0 * SLAB, (c0 + nch) * SLAB)
                    nc.scalar.activation(atmp[:, hs], sallh[:, hs], AF.Sqrt)

                def aff_exp(c0, nch):
                    hs = slice(c0 * SLAB, (c0 + nch) * SLAB)
                    nc.scalar.activation(affh[:, hs], atmp[:, hs], AF.Exp,
                                         scale=-1.0)

                def stage1(c0, nch):
                    hs = slice(c0 * SLAB, (c0 + nch) * SLAB)
                    h2 = slice(c0 * 2 * SLAB, (c0 + nch) * 2 * SLAB)

                    def rep(t):
                        return t[:, hs].rearrange(
                            "p (c n) -> p c n", c=nch).unsqueeze(2) \
                            .to_broadcast((P, nch, 2, SLAB))

                    def v4(t):
                        return t[:, h2].rearrange("p (c b n) -> p c b n",
                                                  c=nch, b=2)

                    nc.vector.tensor_tensor(v4(bdXY), v4(dXYall), rep(dhall),
                                            ALU.mult)
                    nc.vector.tensor_tensor(bdXY2[:, h2], bdXY[:, h2],
                                            dXYall[:, h2], ALU.mult)
                    nc.vector.tensor_tensor(d2all[:, hs], dhall[:, hs],
                                            dhall[:, hs], ALU.mult)
                    nc.vector.tensor_tensor(d3all[:, hs], d2all[:, hs],
                                            dhall[:, hs], ALU.mult)

                def stage2(c0, nch):
                    hs = slice(c0 * SLAB, (c0 + nch) * SLAB)
                    h2 = slice(c0 * 2 * SLAB, (c0 + nch) * 2 * SLAB)

                    def rep(t):
                        return t[:, hs].rearrange(
                            "p (c n) -> p c n", c=nch).unsqueeze(2) \
                            .to_broadcast((P, nch, 2, SLAB))

                    def v4(t):
                        return t[:, h2].rearrange("p (c b n) -> p c b n",
                                                  c=nch, b=2)

                    nc.vector.tensor_tensor(bAall[:, hs], affh[:, hs],
                                            d2all[:, hs], ALU.mult)
                    nc.vector.tensor_tensor(v4(bAdXY), v4(dXYall), rep(bAall),
                                            ALU.mult)
                    nc.vector.tensor_tensor(bAdXY2[:, h2], bAdXY[:, h2],
                                            dXYall[:, h2], ALU.mult)
                    nc.vector.tensor_tensor(bAd2[:, hs], bAall[:, hs],
                                            d2all[:, hs], ALU.mult)

                def r_surf(m, c):
                    cs = slice(c * SLAB, (c + 1) * SLAB)
                    cx = slice(c * 2 * SLAB, c * 2 * SLAB + SLAB)
                    cy = slice(c * 2 * SLAB + SLAB, (c + 1) * 2 * SLAB)
                    return [bdXY[:, cx], bdXY[:, cy], bdXY2[:, cx],
                            bdXY2[:, cy], d3all[:, cs], d2all[:, cs]][m]

                def h_surf(m, c):
                    cs = slice(c * SLAB, (c + 1) * SLAB)
                    cx = slice(c * 2 * SLAB, c * 2 * SLAB + SLAB)
                    cy = slice(c * 2 * SLAB + SLAB, (c + 1) * 2 * SLAB)
                    return [bAall[:, cs], bAdXY[:, cx], bAdXY[:, cy],
                            bAdXY2[:, cx], bAdXY2[:, cy], bAd2[:, cs]][m]

                def r_mms(c0, nch):
                    # m-major over the group's chunks, in band-arrival order
                    for m in (0, 1, 2, 3, 5, 4):
                        for c in range(c0, c0 + nch):
                            ro = c * ACC_CW + NHB * 21
                            nc.tensor.matmul(
                                accR[:],
                                accst[:, ro + m * 6 : ro + (m + 1) * 6],
                                r_surf(m, c),
                                start=(m == 0 and c == 0),
                                stop=(m == 4 and c == KC - 1))

                def h_mms(c0, nch):
                    for m in (0, 1, 2, 3, 4, 5):
                        for c in range(c0, c0 + nch):
                            co = c * ACC_CW
                            nc.tensor.matmul(
                                accH[:],
                                accst[:, co + m * 21 : co + (m + 1) * 21],
                                h_surf(m, c),
                                start=(m == 0 and c == 0),
                                stop=(m == 5 and c == KC - 1))

                for c in range(4):
                    pass_s(c)
                aff_sqrt(0, 4)
                for c in range(4, 8):
                    pass_s(c)
                aff_sqrt(4, 4)
                aff_exp(0, 4)
                aff_exp(4, 4)
                for c in range(4):
                    pass_zxy(c)
                stage1(0, 4)
                r_mms(0, 4)
                stage2(0, 4)
                pass_zxy(4)
                pass_zxy(5)
                pass_zxy(6)
                pass_zxy(7)
                h_mms(0, 4)
                stage1(4, 4)
                r_mms(4, 4)
                stage2(4, 4)
                h_mms(4, 4)

            # ---------------- solve / exp map / compose -----------------
            with tc.tile_pool(name="post", bufs=2) as qp, \
                 tc.tile_pool(name="post_ps", bufs=2, space="PSUM") as qps:
                acc_sb = qp.tile([64, SLAB], F32)
                # rhs accumulator closes before Hm: copy + expand it first
                nc.scalar.copy(acc_sb[32:38, :], accR[:])

                # pre-zeroed divide targets: f12_j keeps its col j at 0
                f12s = qp.tile([P, 72], F32)
                nc.gpsimd.memset(f12s[:], 0.0)
                upd = qp.tile([P, 84], F32)

                # expand entry rows -> [anchor, 6x7 augmented] per half
                hb = qp.tile([P, 2 * NE], F32)  # ih-major: [0:48]=ih0, [48:96]=ih1
                hb_ps = [qps.tile([P, NE], F32, name=f"hbps{ih}", tag="hbps")
                         for ih in range(2)]
                for ih in range(2):
                    nc.tensor.matmul(hb_ps[ih][:],
                                     acc_sb[32:38, ih * P : (ih + 1) * P],
                                     misc[32:38, 56:104],
                                     start=True, stop=False)
                nc.scalar.copy(acc_sb[0:21, :], accH[:])
                for ih in range(2):
                    nc.tensor.matmul(hb_ps[ih][:],
                                     acc_sb[0:21, ih * P : (ih + 1) * P],
                                     misc[0:21, 56:104],
                                     start=False, stop=True)
                    nc.scalar.copy(hb[:, ih * NE : (ih + 1) * NE], hb_ps[ih][:])

                # ---------------- Gauss-Jordan (both halves packed) --------
                def hbv(sl):
                    return hb[:].rearrange("p (i e) -> p i e", i=2)[:, :, sl]
                rpiv = qp.tile([P, 2], F32)
                for j in range(6):
                    w = 7 - j     # columns < j are never read again
                    f12v = f12s[:, 12 * j : 12 * (j + 1)].rearrange(
                        "p (i r) -> p i r", i=2)
                    col = hbv(slice(j, 42, 7))
                    nc.vector.reciprocal(rpiv[:], hb[:, 8 * j : 2 * NE : NE])
                    if j > 0:
                        nc.vector.tensor_tensor(
                            f12v[:, :, 0:j], col[:, :, 0:j],
                            rpiv[:].to_broadcast((P, 2, j)), ALU.mult)
                    if j < 5:
                        nc.vector.tensor_tensor(
                            f12v[:, :, j + 1 : 6], col[:, :, j + 1 : 6],
                            rpiv[:].to_broadcast((P, 2, 5 - j)), ALU.mult)
                    updv = upd[:, 0 : 12 * w].rearrange("p (i r c) -> p i r c",
                                                        r=6, c=w)
                    nc.vector.tensor_tensor(
                        updv, f12v.to_broadcast((P, 2, 6, w)),
                        hbv(slice(7 * j + j, 7 * j + 7)).unsqueeze(2)
                            .to_broadcast((P, 2, 6, w)),
                        ALU.mult)
                    hview = hbv(slice(0, 42)).rearrange(
                        "p i (r c) -> p i r c", c=7)[:, :, :, j:7]
                    nc.vector.tensor_tensor(hview, hview, updv, ALU.subtract)
                delta = qp.tile([P, 12], F32)
                dinv = qp.tile([P, 12], F32)
                deltav = delta[:].rearrange("p (i r) -> p i r", i=2)
                dinvv = dinv[:].rearrange("p (i r) -> p i r", i=2)
                nc.vector.reciprocal(dinvv, hbv(slice(0, 42, 8)))
                nc.vector.tensor_tensor(deltav, hbv(slice(6, 42, 7)),
                                        dinvv, ALU.mult)

                # ------------- exp map coefficients via Taylor in th^2 -----
                K1 = misc[:, 104:107]
                K0 = misc[:, 107:110]
                K2 = misc[:, 110:113]
                K3 = misc[:, 113:116]
                wsq = qp.tile([P, 6], F32)
                th2 = qp.tile([P, 2], F32)
                wv = deltav[:, :, 3:6]
                vb = deltav[:, :, 0:3]
                wsqv = wsq[:].rearrange("p (i r) -> p i r", i=2)
                nc.vector.tensor_tensor(wsqv, wv, wv, ALU.mult)
                nc.vector.tensor_reduce(th2[:], wsqv, AX.X, ALU.add)
                tu2 = qp.tile([P, 2], F32)
                tu3 = qp.tile([P, 2], F32)
                nc.vector.tensor_tensor(tu2[:], th2[:], th2[:], ALU.mult)
                nc.vector.tensor_tensor(tu3[:], tu2[:], th2[:], ALU.mult)
                # abc [P, 6] = (coef, ih); series in th^2 with constant tiles
                abc = qp.tile([P, 6], F32)
                t6 = qp.tile([P, 6], F32)
                tm6 = qp.tile([P, 6], F32)
                t6v = t6[:].rearrange("p (k i) -> p k i", k=3)
                abcv = abc[:].rearrange("p (k i) -> p k i", k=3)
                tm6v = tm6[:].rearrange("p (k i) -> p k i", k=3)

                def krep(t):
                    return t.unsqueeze(2).to_broadcast((P, 3, 2))

                def trep(t):
                    return t.unsqueeze(1).to_broadcast((P, 3, 2))

                nc.vector.tensor_tensor(t6v, trep(th2[:]), krep(K1), ALU.mult)
                nc.vector.tensor_tensor(t6v, t6v, krep(K0), ALU.add)
                nc.vector.tensor_tensor(tm6v, trep(tu2[:]), krep(K2), ALU.mult)
                nc.vector.tensor_tensor(abcv, t6v, tm6v, ALU.add)
                nc.vector.tensor_tensor(tm6v, trep(tu3[:]), krep(K3), ALU.mult)
                nc.vector.tensor_tensor(abcv, abcv, tm6v, ALU.add)

                # ------- fused R/V pieces: diag/plus/minus groups ---------
                u3 = qp.tile([P, 6], F32)       # (ih, r): w_r^2 - th^2
                u3v = u3[:].rearrange("p (i r) -> p i r", i=2)
                nc.vector.tensor_tensor(
                    u3v, wsqv, th2[:].unsqueeze(2).to_broadcast((P, 2, 3)),
                    ALU.subtract)
                # wcat [P, 18] = (coef A/B/C, ih, r): coef_k * w_r
                wcat = qp.tile([P, 18], F32)
                wcatv = wcat[:].rearrange("p (k i r) -> p k i r", k=3, i=2)
                nc.vector.tensor_tensor(
                    wcatv, wv.unsqueeze(1).to_broadcast((P, 3, 2, 3)),
                    abcv.unsqueeze(3).to_broadcast((P, 3, 2, 3)), ALU.mult)
                # dBC [P, 12] = (B/C, ih, r): u3 * (B|C)
                dBC = qp.tile([P, 12], F32)
                dBCv = dBC[:].rearrange("p (k i r) -> p k i r", k=2, i=2)
                nc.vector.tensor_tensor(
                    dBCv, u3v.unsqueeze(1).to_broadcast((P, 2, 2, 3)),
                    abc[:, 2:6].rearrange("p (k i) -> p k i", k=2)
                        .unsqueeze(3).to_broadcast((P, 2, 2, 3)), ALU.mult)
                # qcb [P, 12] = (q in (02,01,12), B/C, ih)
                qcb = qp.tile([P, 12], F32)
                wcatBC = wcat[:, 6:18].rearrange("p (k i r) -> p k i r",
                                                 k=2, i=2)

                def wcol(r):
                    return delta[:, 3 + r : 12 : 6]

                for q, (rw, wc) in enumerate(((0, 2), (0, 1), (1, 2))):
                    nc.vector.tensor_tensor(
                        qcb[:, q * 4 : (q + 1) * 4].rearrange(
                            "p (k i) -> p k i", k=2).unsqueeze(3),
                        wcatBC[:, :, :, rw : rw + 1],
                        wcol(wc).unsqueeze(1).unsqueeze(3)
                            .to_broadcast((P, 2, 2, 1)), ALU.mult)
                # D = 1 + dBC   [P, 12] (B/C, ih, r)
                D = qp.tile([P, 12], F32)
                nc.vector.tensor_scalar(D[:], dBC[:], 1.0, None, ALU.add)
                # doubled tiles for cyclic slicing (on Scalar, off V path)
                wcatD = qp.tile([P, 24], F32)   # (A/B hat, ih, r r)
                nc.scalar.copy(
                    wcatD[:].rearrange("p (h i d r) -> p h i d r",
                                       h=2, i=2, d=2),
                    wcat[:, 0:12].rearrange("p (h i r) -> p h i r", h=2, i=2)
                        .unsqueeze(3).to_broadcast((P, 2, 2, 2, 3)))
                qcbD = qp.tile([P, 24], F32)    # (q q, B/C, ih)
                nc.scalar.copy(
                    qcbD[:].rearrange("p (d q k i) -> p d q k i",
                                      d=2, q=3, k=2),
                    qcb[:].rearrange("p (q k i) -> p q k i", q=3, k=2)
                        .unsqueeze(1).to_broadcast((P, 2, 3, 2, 2)))
                wcatDv = wcatD[:].rearrange("p (h i r) -> p h i r", h=2, i=2)
                qcbDv = qcbD[:].rearrange("p (q k i) -> p k i q", q=6, k=2)
                # PL/MI [P, 12] = (R/V, ih, r)
                PL = qp.tile([P, 12], F32)
                MI = qp.tile([P, 12], F32)
                PLv = PL[:].rearrange("p (k i r) -> p k i r", k=2, i=2)
                MIv = MI[:].rearrange("p (k i r) -> p k i r", k=2, i=2)
                nc.vector.tensor_tensor(
                    PLv, qcb[:].rearrange("p (q k i) -> p k i q", q=3, k=2),
                    wcatDv[:, :, :, 1:4], ALU.add)
                nc.vector.tensor_tensor(
                    MIv, qcbDv[:, :, :, 1:4],
                    wcatDv[:, :, :, 2:5], ALU.subtract)

                # ----- translation chain on GpSimd (parallel with compose)
                vD = qp.tile([P, 12], F32)      # (ih, r r)
                nc.gpsimd.tensor_copy(
                    vD[:].rearrange("p (i d r) -> p i d r", i=2, d=2),
                    vb.unsqueeze(2).to_broadcast((P, 2, 2, 3)))
                vDv = vD[:].rearrange("p (i r) -> p i r", i=2)
                ta = qp.tile([P, 6], F32)
                tb = qp.tile([P, 6], F32)
                tcx = qp.tile([P, 6], F32)
                tav = ta[:].rearrange("p (i r) -> p i r", i=2)
                tbv = tb[:].rearrange("p (i r) -> p i r", i=2)
                tcv = tcx[:].rearrange("p (i r) -> p i r", i=2)
                nc.gpsimd.tensor_tensor(
                    tav, D[:, 6:12].rearrange("p (i r) -> p i r", i=2),
                    vb, ALU.mult)
                nc.gpsimd.tensor_tensor(
                    tbv, PL[:, 6:12].rearrange("p (i r) -> p i r", i=2),
                    vDv[:, :, 2:5], ALU.mult)
                nc.gpsimd.tensor_tensor(
                    tcv, MI[:, 6:12].rearrange("p (i r) -> p i r", i=2),
                    vDv[:, :, 1:4], ALU.mult)
                nc.gpsimd.tensor_tensor(tav, tav, tbv, ALU.add)
                nc.gpsimd.tensor_tensor(tav, tav, tcv, ALU.add)
                o4 = qp.tile([P, 24], F32)
                o4v = o4[:].rearrange("p (i r c) -> p i r c", i=2, r=3)
                nc.gpsimd.tensor_tensor(
                    o4v, tav.unsqueeze(3).to_broadcast((P, 2, 3, 4)),
                    misc[:, 48:56].rearrange("p (i c) -> p i c", i=2)
                        .unsqueeze(2).to_broadcast((P, 2, 3, 4)), ALU.mult)

                # ----- compose out = dT @ Tmat via term accumulation ------
                TmD = misc[:, 0:48].rearrange("p (i d c) -> p i d c",
                                              i=2, d=6)
                Ob = qp.tile([P, 32], F32)
                # bottom row is a Tmat passthrough
                nc.scalar.copy(Ob[:, 12:16], misc[:, 48:52])
                nc.scalar.copy(Ob[:, 28:32], misc[:, 52:56])
                o1 = qp.tile([P, 24], F32)
                o2 = qp.tile([P, 24], F32)
                o1v = o1[:].rearrange("p (i r c) -> p i r c", i=2, r=3)
                o2v = o2[:].rearrange("p (i r c) -> p i r c", i=2, r=3)

                def dpm_rep(t):
                    return t[:, 0:6].rearrange("p (i r) -> p i r", i=2) \
                        .unsqueeze(3).to_broadcast((P, 2, 3, 4))

                nc.vector.tensor_tensor(o1v, dpm_rep(D), TmD[:, :, 0:3, :],
                                        ALU.mult)
                nc.vector.tensor_tensor(o2v, dpm_rep(PL), TmD[:, :, 2:5, :],
                                        ALU.mult)
                nc.vector.tensor_tensor(o1v, o1v, o2v, ALU.add)
                nc.vector.tensor_tensor(o2v, dpm_rep(MI), TmD[:, :, 1:4, :],
                                        ALU.mult)
                nc.vector.tensor_tensor(o1v, o1v, o2v, ALU.add)
                obv = Ob[:].rearrange("p (i r c) -> p i r c", i=2, r=4)
                nc.vector.tensor_tensor(obv[:, :, 0:3, :], o1v, o4v, ALU.add)
                nc.sync.dma_start(out_d[:], Ob[:])

    nc.compile()
    return nc


def _q16(x):
    return np.asarray(x, np.float16).astype(np.float64)


def prep_inputs(embeddings, revisions, weights, depth, pix_T_camXs, Tmat):
    f6 = np.float64
    emb = _q16(np.asarray(embeddings, f6).reshape(B, C, N))
    rev = np.asarray(revisions, f6).reshape(B, 3, N)
    wgt = np.asarray(weights, f6).reshape(B, 3, N)
    dep = np.asarray(depth, f6).reshape(B, N)
    pix = np.asarray(pix_T_camXs, f6)
    tm = np.asarray(Tmat, f6).reshape(B, N, 16)

    ys, xs = np.meshgrid(np.arange(H, dtype=f6), np.arange(W, dtype=f6),
                         indexing="ij")
    u = xs.reshape(-1)
    v = ys.reshape(-1)

    in_maps = []
    per_batch = []
    for b in range(B):
        fx, fy, x0, y0 = pix[b, 0, 0], pix[b, 1, 1], pix[b, 0, 2], pix[b, 1, 2]
        z = _q16(dep[b])
        X = _q16((u - x0) * dep[b] / fx)
        Y = _q16((v - y0) * dep[b] / fy)
        T0 = tm[b].reshape(N, 4, 4)
        dR = _q16(T0[:, :3, :3] - np.eye(3))   # rotations are near identity
        R = np.eye(3) + dR
        t = _q16(T0[:, :3, 3])
        xyz = np.stack([X, Y, z], -1)
        TjXj = np.einsum("kpq,kq->kp", R, xyz) + t
        w0, w1, w2 = wgt[b, 0], wgt[b, 1], wgt[b, 2]
        r0, r1, r2 = rev[b, 0], rev[b, 1], rev[b, 2]
        # fold the revisions into the self-projection constants so the
        # residual is exactly fx*dX' / fy*dY' / dD' (no d / d^2 bands)
        Xkk = TjXj[:, 0] / TjXj[:, 2] + r0 / fx
        Ykk = TjXj[:, 1] / TjXj[:, 2] + r1 / fy
        dkk = 1.0 / TjXj[:, 2] + r2
        on, zn = np.ones(N), np.zeros(N)
        JT0 = np.stack([on, zn, zn, zn, -z, Y], -1)
        JT1 = np.stack([zn, on, zn, z, zn, -X], -1)
        JT2 = np.stack([zn, zn, on, -Y, X, zn], -1)
        G0 = JT0 - Xkk[:, None] * JT2
        G1 = JT1 - Ykk[:, None] * JT2

        def outer(a, bb):
            return np.einsum("kp,kq->kpq", a, bb)

        P00 = outer(G0, G0)
        P11 = outer(G1, G1)
        P22 = outer(JT2, JT2)
        P02 = outer(G0, JT2) + outer(JT2, G0)
        P12 = outer(G1, JT2) + outer(JT2, G1)
        wfx = (w0 * fx * fx)[:, None, None]
        wfy = (w1 * fy * fy)[:, None, None]
        SH = [wfx * P00 + wfy * P11, -wfx * P02, -wfy * P12,
              wfx * P22, wfy * P22, w2[:, None, None] * P22]
        # rhs surfaces: ddX, ddY, ddX2, ddY2, d3, d2 (dkk folded into d2)
        SR = [fx * fx * G0,
              fy * fy * G1,
              -fx * fx * JT2,
              -fy * fy * JT2,
              -JT2,
              dkk[:, None] * JT2]
        lam = 1.0 / (fx * fx)
        # acc stationary [128, KC*ACC_CW]; partition p of chunk c is k=c*128+p
        accst = np.zeros((P, KC * ACC_CW), f6)
        for c in range(KC):
            ks = slice(c * P, (c + 1) * P)
            co = c * ACC_CW
            for m, S in enumerate(SH):
                for ei, (p_, q_) in enumerate(HTRI):
                    accst[:, co + m * 21 + ei] = S[ks, p_, q_] * lam
            ro = co + NHB * 21
            for m, V in enumerate(SR):
                accst[:, ro + m * 6 : ro + (m + 1) * 6] = V[ks] * lam
        # geometry stationary [32, N]
        stat = np.zeros((GR, N), f6)
        stat[0:16] = -2.0 * emb[b]
        stat[16], stat[17], stat[18], stat[19] = X, Y, z, 1.0
        stat[20:24] = _q16(Xkk[None] * stat[16:20])
        stat[24:28] = _q16(Ykk[None] * stat[16:20])
        stat[28] = _q16((emb[b] ** 2).sum(0))
        stat[29] = _q16(X - Xkk * z)
        stat[30] = _q16(Y - Ykk * z)
        stat[31] = z
        per_batch.append(dict(stat=stat, accst=accst,
                              emb=emb[b], dR=dR, t=t))

    # combined expander [64 rows, 48]: rows 0:21 Hm entries, 32:38 rhs
    cmb = np.zeros((P, 48), np.float32)
    for ei, (p_, q_) in enumerate(HTRI):
        cmb[ei, p_ * 7 + q_] = 1.0
        if p_ != q_:
            cmb[ei, q_ * 7 + p_] = 1.0
    for p_ in range(6):
        cmb[32 + p_, p_ * 7 + 6] = 1.0

    for core in range(NCORES):
        b = core // 4
        s0 = (core % 4) * SLAB
        pb = per_batch[b]
        dRs = pb["dR"][s0 : s0 + SLAB]
        ts = pb["t"][s0 : s0 + SLAB]
        # moving operand [32, 4*SLAB]: X' | Y' | Z | s blocks
        mov = np.zeros((GR, 4 * SLAB), f6)
        for blk, row in ((0, 0), (1, 1), (2, 2)):
            mov[16:19, blk * SLAB : (blk + 1) * SLAB] = dRs[:, row, :].T
            mov[19, blk * SLAB : (blk + 1) * SLAB] = ts[:, row]
        mov[20:23, 0:SLAB] = -dRs[:, 2, :].T
        mov[23, 0:SLAB] = -ts[:, 2]
        mov[24:27, SLAB : 2 * SLAB] = -dRs[:, 2, :].T
        mov[27, SLAB : 2 * SLAB] = -ts[:, 2]
        mov[29, 0:SLAB] = 1.0
        mov[30, SLAB : 2 * SLAB] = 1.0
        mov[31, 2 * SLAB : 3 * SLAB] = 1.0
        ei_ = pb["emb"][:, s0 : s0 + SLAB]
        mov[0:16, 3 * SLAB : 4 * SLAB] = ei_
        mov[19, 3 * SLAB : 4 * SLAB] = _q16((ei_ ** 2).sum(0))
        mov[28, 3 * SLAB : 4 * SLAB] = 1.0

        geom = np.concatenate([pb["stat"], mov], 1)

        misc = np.zeros((P, 128), np.float32)
        tms = np.asarray(tm[b][s0 : s0 + SLAB], np.float32)
        # TmD: doubled top-3 rows of Tmat for cyclic row slicing
        for i_h in range(2):
            tmi = tms[i_h * P : (i_h + 1) * P]
            for dd in range(6):
                misc[:, i_h * 24 + dd * 4 : i_h * 24 + (dd + 1) * 4] = \
                    tmi[:, (dd % 3) * 4 : (dd % 3 + 1) * 4]
            misc[:, 48 + i_h * 4 : 52 + i_h * 4] = tmi[:, 12:16]
        misc[:, 56:104] = cmb
        misc[:, 104:107] = [-1.0 / 6.0, -1.0 / 24.0, -1.0 / 120.0]
        misc[:, 107:110] = [1.0, 0.5, 1.0 / 6.0]
        misc[:, 110:113] = [1.0 / 120.0, 1.0 / 720.0, 1.0 / 5040.0]
        misc[:, 113:116] = [-1.0 / 5040.0, -1.0 / 40320.0, -1.0 / 362880.0]

        in_maps.append({
            "geom": np.ascontiguousarray(geom, np.float16),
            "accst": np.ascontiguousarray(pb["accst"], np.float16),
            "misc": np.ascontiguousarray(misc),
        })
    return in_maps


def gather_output(results):
    full = np.empty((B, N, 16), dtype=np.float32)
    for core in range(NCORES):
        b = core // 4
        s0 = (core % 4) * SLAB
        out = results[core]["out"]
        full[b, s0 : s0 + P] = out[:, 0:16]
        full[b, s0 + P : s0 + SLAB] = out[:, 16:32]
    return full.reshape(B, H, W, 4, 4)


_NC_CACHE = {}


def kernel(**inputs):
    if "nc" not in _NC_CACHE:
        _NC_CACHE["nc"] = build_nc()
    nc = _NC_CACHE["nc"]
    in_maps = prep_inputs(**inputs)
    res = run_bass_kernel_spmd(nc, in_maps, core_ids=list(range(NCORES)))
    return gather_output(res.results)
